# revision 19
# baseline (speedup 1.0000x reference)
"""Trainium2 Bass kernel: BitNet-style BasicBlock.

  out = silu(bn2(conv2(silu(bn1(conv1(x) )))) + x)

with ternary-quantized 3x3 conv weights (s = median|W|, Wq = clamp(round(W/s),-1,1)*s),
inference-mode BN folded into per-channel scale/bias.

Strategy:
  - Data-parallel over batch: 32 images -> 4 per NeuronCore (8 cores), weights replicated.
  - Conv3x3 via 9 shifted matmuls on a zero-padded 34x34 image layout. Weights enter the
    PE array as exact ternary {-1,0,+1} bf16; the quantization scale and BN affine are
    applied in f32 epilogues, so the only precision loss is bf16 rounding of activations.
  - Per output-channel-tile (128) and 16-row chunk, the moving operand is a strided
    [c, 16 rows, 32 cols] view (N=512, one PSUM bank); 18 matmuls (9 taps x 2 input
    channel tiles, K=128 each) accumulate into the bank at the PE stream roofline.
  - Epilogue conv1: ScalarE Silu(psum*sc1 + bb1) -> bf16 into the padded conv2 input.
    Epilogue conv2: VectorE (psum*sc2 + identity), ScalarE Silu(.+bb2) -> f32 out.
  - DMA queues: activations on qSync, weights on qScalar, stores on qGpSimd.
"""

import numpy as np

_BN_EPS = 1e-5

_PW = 34              # padded width/height
_PAD = _PW * _PW      # 1156
# output row chunks in padded coords: (first padded row, n rows)
_CHUNKS = [(1, 16), (17, 16)]

LAST_RESULTS = None   # BassKernelResults of the most recent run (for test harnesses)


def _quantize(w):
    """Ternary matrix (exact in bf16) + scale, matching the jax reference."""
    w = np.asarray(w, np.float32)
    s = np.median(np.abs(w))
    t = np.clip(np.round(w / s), -1.0, 1.0).astype(np.float32)
    return t, np.float32(s)


def _fold_bn(s, b, g, be, m, v):
    """BN(conv*s + b) = psum * sc + bb  (psum = ternary conv result)."""
    inv = g.astype(np.float32) / np.sqrt(v.astype(np.float32) + _BN_EPS)
    sc = (s * inv).astype(np.float32)
    bb = ((b - m) * inv + be).astype(np.float32)
    return sc, bb


def _prep_weights(t):
    """[O,C,3,3] ternary -> [ct, c128, tap, ot, o128] bf16 (lhsT layout)."""
    import ml_dtypes

    arr = t.transpose(1, 2, 3, 0).reshape(2, 128, 9, 2, 128)
    return np.ascontiguousarray(arr.astype(ml_dtypes.bfloat16))


def _build_program():
    import concourse.bass as bass
    import concourse.tile as tile
    from concourse import bacc, mybir

    f32 = mybir.dt.float32
    bf16 = mybir.dt.bfloat16
    AF = mybir.ActivationFunctionType
    OP = mybir.AluOpType

    nc = bacc.Bacc("TRN2", target_bir_lowering=False, debug=False,
                   enable_asserts=False, num_devices=8)
    x_in = nc.declare_dram_parameter("x_in", [4, 128, 2, 1024], bf16, isOutput=False)
    w1p = nc.declare_dram_parameter("w1p", [2, 128, 9, 2, 128], bf16, isOutput=False)
    w2p = nc.declare_dram_parameter("w2p", [2, 128, 9, 2, 128], bf16, isOutput=False)
    scl = nc.declare_dram_parameter("scales", [128, 8], f32, isOutput=False)
    y_out = nc.declare_dram_parameter("y_out", [4, 2, 128, 1024], f32, isOutput=True)

    with tile.TileContext(nc) as tc:
        from contextlib import ExitStack

        with ExitStack() as ctx:
            psum = ctx.enter_context(tc.tile_pool(name="psum", bufs=8, space="PSUM"))
            zpool = ctx.enter_context(tc.tile_pool(name="zp", bufs=4))
            const = ctx.enter_context(tc.tile_pool(name="const", bufs=1))

            w_sb = {}
            x_id = {}
            xp = {}      # (conv_input_index 0/1, img, ct) -> padded bf16 tile
            outs = {}

            sc_sb = const.tile([128, 8], f32, tag="scales", name="scales_sb")

            def load_w(ci, wdram):
                for ct in range(2):
                    t = const.tile([128, 9, 2, 128], bf16, tag=f"w{ci}_{ct}",
                                   name=f"w{ci}_{ct}")
                    nc.scalar.dma_start(out=t[:], in_=wdram[ct])
                    w_sb[ci, ct] = t

            def load_x(img, eng=None):
                t = const.tile([128, 2, 1024], bf16, tag=f"xid_{img}",
                               name=f"xid_{img}")
                (eng or nc.sync).dma_start(out=t[:], in_=x_in[img])
                x_id[img] = t

            def zero_borders(t):
                # rows 0 and 33, plus cols 0 and 33 of rows 1..32 (one 2D AP)
                nc.vector.memset(t[:, 0:_PW], 0.0)
                nc.vector.memset(t[:, _PAD - _PW : _PAD], 0.0)
                base = t[:]
                colpair = bass.AP(
                    tensor=base.tensor,
                    offset=base.offset + _PW,
                    ap=[list(base.ap[0]), [_PW, 32], [33, 2]],
                )
                nc.vector.memset(colpair, 0.0)

            def prep(img):
                # build conv1 input (pad + cast to bf16); pre-zero conv2 input pads
                for ct in range(2):
                    t = const.tile([128, _PAD], bf16, tag=f"xp1_{img}_{ct}",
                                   name=f"xp1_{img}_{ct}")
                    zero_borders(t)
                    v = t[:].rearrange("p (y x) -> p y x", x=_PW)
                    xv = x_id[img][:, ct, :].rearrange("p (y x) -> p y x", x=32)
                    nc.vector.tensor_copy(v[:, 1:33, 1:33], xv)
                    xp[0, img, ct] = t
                    t2 = const.tile([128, _PAD], bf16, tag=f"xp2_{img}_{ct}",
                                    name=f"xp2_{img}_{ct}")
                    zero_borders(t2)
                    xp[1, img, ct] = t2

            def conv(img, ci, interleave=False, chunks=_CHUNKS, split_last_store=False):
                if ci == 1:
                    for ot in range(2):
                        outs[img, ot] = const.tile(
                            [128, 1024], f32, tag=f"out_{img}_{ot}",
                            name=f"out_{img}_{ot}",
                        )
                src_views = [
                    xp[ci, img, ct][:].rearrange("p (y x) -> p y x", x=_PW)
                    for ct in range(2)
                ]
                groups = []
                for ot in range(2):
                    for r0, nr in chunks:
                        ps = psum.tile([128, 512], f32, tag="ps", name="ps")
                        groups.append((ot, r0, nr, ps))

                def emit_mms(group_list):
                    for ct in range(2):
                        for ot, r0, nr, ps in group_list:
                            for tap in range(9):
                                dy, dx = tap // 3 - 1, tap % 3 - 1
                                rhs = src_views[ct][
                                    :, r0 + dy : r0 + dy + nr, 1 + dx : 33 + dx
                                ]
                                nc.tensor.matmul(
                                    ps[:, : nr * 32],
                                    w_sb[ci, ct][:, tap, ot, :],
                                    rhs,
                                    start=(ct == 0 and tap == 0),
                                    stop=(ct == 1 and tap == 8),
                                )

                if interleave:
                    # ct-outer across all 4 groups: gives the ct=1 data more
                    # time to arrive (used for the first conv only; it bunches
                    # the group completions, which is bad at the kernel tail).
                    emit_mms(groups)
                else:
                    for g in groups:
                        emit_mms([g])
                for ot, r0, nr, ps in groups:
                    n = nr * 32
                    pss = ps[:, :n]
                    if ci == 0:
                        dst = xp[1, img, ot][:].rearrange(
                            "p (y x) -> p y x", x=_PW
                        )
                        psv = pss.rearrange("p (y x) -> p y x", x=32)
                        nc.scalar.activation(
                            dst[:, r0 : r0 + nr, 1:33],
                            psv,
                            AF.Silu,
                            bias=sc_sb[:, 2 + ot : 3 + ot],
                            scale=sc_sb[:, ot : ot + 1],
                        )
                    else:
                        z = zpool.tile([128, 512], f32, tag="z", name="z")
                        zz = z[:, :n]
                        nc.vector.scalar_tensor_tensor(
                            out=zz,
                            in0=pss,
                            scalar=sc_sb[:, 4 + ot : 5 + ot],
                            in1=x_id[img][:, ot, (r0 - 1) * 32 : (r0 - 1 + nr) * 32],
                            op0=OP.mult,
                            op1=OP.add,
                        )
                        lo = (r0 - 1) * 32
                        nc.scalar.activation(
                            outs[img, ot][:, lo : lo + n],
                            zz,
                            AF.Silu,
                            bias=sc_sb[:, 6 + ot : 7 + ot],
                        )
                        is_last = (ot, r0, nr, ps) == groups[-1]
                        if split_last_store and is_last:
                            nc.sync.dma_start(
                                out=y_out[img, ot, :64, lo : lo + n],
                                in_=outs[img, ot][:64, lo : lo + n],
                            )
                            nc.scalar.dma_start(
                                out=y_out[img, ot, 64:, lo : lo + n],
                                in_=outs[img, ot][64:, lo : lo + n],
                            )
                        else:
                            eng = nc.sync if (r0 == 1) else nc.scalar
                            eng.dma_start(
                                out=y_out[img, ot, :, lo : lo + n],
                                in_=outs[img, ot][:, lo : lo + n],
                            )

            # critical-path-first emission: x(0) + conv1 weights first.
            # Queues: sync (HWDGE), scalar (HWDGE), gpsimd (SWDGE).
            # Critical loads: x0 (bf16, one full-packet DMA) on sync;
            # conv1 weights on scalar; scales (tiny packets) on gpsimd.
            load_x(0)
            load_w(0, w1p)
            # scales is 128 partitions x 32B = 128 tiny DMA packets (~4us of
            # queue time) -> keep it off the weight/activation critical queues
            nc.gpsimd.dma_start(out=sc_sb[:], in_=scl[:])

            # PE warm-up: trip the HAM clock-gate during the DMA wait so the
            # first real matmuls run at 2.4 GHz.
            warm = const.tile([128, 128], bf16, tag="warm", name="warm")
            nc.vector.memset(warm[:], 0.0)
            wps = psum.tile([128, 512], f32, tag="ps", name="wps")
            # ~5.4us of continuous cold PE activity: trips the free-running
            # HAM window so the real matmuls start at 2.4 GHz.
            for k in range(50):
                nc.tensor.matmul(wps[:, :128], warm[:], warm[:],
                                 start=(k == 0), stop=(k == 49))

            prep(0)
            load_x(1)
            load_w(1, w2p)
            load_x(2)
            load_x(3)
            prep(1)
            conv(0, 0, interleave=True)
            prep(2)
            conv(1, 0)
            conv(0, 1)
            prep(3)
            conv(2, 0)
            conv(1, 1)
            conv(3, 0)
            conv(2, 1)
            conv(3, 1)

    nc.finalize()
    return nc


_PROGRAM = None


def kernel(x, w1, b1, g1, be1, m1, v1, w2, b2, g2, be2, m2, v2):
    global LAST_RESULTS, _PROGRAM
    from concourse.bass_utils import run_bass_kernel_spmd

    import ml_dtypes

    x = np.asarray(x, np.float32)
    t1, s1 = _quantize(np.asarray(w1, np.float32))
    t2, s2 = _quantize(np.asarray(w2, np.float32))
    sc1, bb1 = _fold_bn(s1, np.asarray(b1), np.asarray(g1), np.asarray(be1),
                        np.asarray(m1), np.asarray(v1))
    sc2, bb2 = _fold_bn(s2, np.asarray(b2), np.asarray(g2), np.asarray(be2),
                        np.asarray(m2), np.asarray(v2))
    w1d = _prep_weights(t1)
    w2d = _prep_weights(t2)
    scales = np.ascontiguousarray(
        np.stack(
            [sc1[:128], sc1[128:], bb1[:128], bb1[128:],
             sc2[:128], sc2[128:], bb2[:128], bb2[128:]],
            axis=1,
        ).astype(np.float32)
    )

    if _PROGRAM is None:
        _PROGRAM = _build_program()
    nc = _PROGRAM

    n_cores = 8
    in_maps = []
    for i in range(n_cores):
        shard = np.ascontiguousarray(
            x[i * 4 : (i + 1) * 4]
            .reshape(4, 2, 128, 1024)
            .transpose(0, 2, 1, 3)
            .astype(ml_dtypes.bfloat16)
        )
        in_maps.append(
            {"x_in": shard, "w1p": w1d, "w2p": w2d, "scales": scales}
        )

    res = None
    for attempt in range(3):
        try:
            res = run_bass_kernel_spmd(nc, in_maps, list(range(n_cores)))
            break
        except Exception:
            if attempt == 2:
                raise
            import time

            time.sleep(2.0)
    LAST_RESULTS = res
    out = np.concatenate(
        [res.results[i]["y_out"].reshape(4, 256, 32, 32) for i in range(n_cores)],
        axis=0,
    )
    return out.astype(np.float32)


# revision 21
# speedup vs baseline: 1.0014x; 1.0014x over previous
"""Trainium2 Bass kernel: BitNet-style BasicBlock.

  out = silu(bn2(conv2(silu(bn1(conv1(x) )))) + x)

with ternary-quantized 3x3 conv weights (s = median|W|, Wq = clamp(round(W/s),-1,1)*s),
inference-mode BN folded into per-channel scale/bias.

Strategy:
  - Data-parallel over batch: 32 images -> 4 per NeuronCore (8 cores), weights replicated.
  - Conv3x3 via 9 shifted matmuls on a zero-padded 34x34 image layout. Weights enter the
    PE array as exact ternary {-1,0,+1} bf16; the quantization scale and BN affine are
    applied in f32 epilogues, so the only precision loss is bf16 rounding of activations.
  - Per output-channel-tile (128) and 16-row chunk, the moving operand is a strided
    [c, 16 rows, 32 cols] view (N=512, one PSUM bank); 18 matmuls (9 taps x 2 input
    channel tiles, K=128 each) accumulate into the bank at the PE stream roofline.
  - Epilogue conv1: ScalarE Silu(psum*sc1 + bb1) -> bf16 into the padded conv2 input.
    Epilogue conv2: VectorE (psum*sc2 + identity), ScalarE Silu(.+bb2) -> f32 out.
  - DMA queues: activations on qSync, weights on qScalar, stores on qGpSimd.
"""

import numpy as np

_BN_EPS = 1e-5

_PW = 34              # padded width/height
_PAD = _PW * _PW      # 1156
# output row chunks in padded coords: (first padded row, n rows)
_CHUNKS = [(1, 16), (17, 16)]

LAST_RESULTS = None   # BassKernelResults of the most recent run (for test harnesses)


def _quantize(w):
    """Ternary matrix (exact in bf16) + scale, matching the jax reference."""
    w = np.asarray(w, np.float32)
    s = np.median(np.abs(w))
    t = np.clip(np.round(w / s), -1.0, 1.0).astype(np.float32)
    return t, np.float32(s)


def _fold_bn(s, b, g, be, m, v):
    """BN(conv*s + b) = psum * sc + bb  (psum = ternary conv result)."""
    inv = g.astype(np.float32) / np.sqrt(v.astype(np.float32) + _BN_EPS)
    sc = (s * inv).astype(np.float32)
    bb = ((b - m) * inv + be).astype(np.float32)
    return sc, bb


def _prep_weights(t):
    """[O,C,3,3] ternary -> [ct, c128, tap, ot, o128] bf16 (lhsT layout)."""
    import ml_dtypes

    arr = t.transpose(1, 2, 3, 0).reshape(2, 128, 9, 2, 128)
    return np.ascontiguousarray(arr.astype(ml_dtypes.bfloat16))


def _build_program():
    import concourse.bass as bass
    import concourse.tile as tile
    from concourse import bacc, mybir

    f32 = mybir.dt.float32
    bf16 = mybir.dt.bfloat16
    AF = mybir.ActivationFunctionType
    OP = mybir.AluOpType

    nc = bacc.Bacc("TRN2", target_bir_lowering=False, debug=False,
                   enable_asserts=False, num_devices=8)
    x_in = nc.declare_dram_parameter("x_in", [4, 128, 2, 1024], bf16, isOutput=False)
    w1p = nc.declare_dram_parameter("w1p", [2, 128, 9, 2, 128], bf16, isOutput=False)
    w2p = nc.declare_dram_parameter("w2p", [2, 128, 9, 2, 128], bf16, isOutput=False)
    scl = nc.declare_dram_parameter("scales", [128, 8], f32, isOutput=False)
    y_out = nc.declare_dram_parameter("y_out", [4, 2, 128, 1024], f32, isOutput=True)

    with tile.TileContext(nc) as tc:
        from contextlib import ExitStack

        with ExitStack() as ctx:
            psum = ctx.enter_context(tc.tile_pool(name="psum", bufs=8, space="PSUM"))
            zpool = ctx.enter_context(tc.tile_pool(name="zp", bufs=4))
            const = ctx.enter_context(tc.tile_pool(name="const", bufs=1))

            w_sb = {}
            x_id = {}
            xp = {}      # (conv_input_index 0/1, img, ct) -> padded bf16 tile
            outs = {}

            sc_sb = const.tile([128, 8], f32, tag="scales", name="scales_sb")

            def load_w(ci, wdram):
                for ct in range(2):
                    t = const.tile([128, 9, 2, 128], bf16, tag=f"w{ci}_{ct}",
                                   name=f"w{ci}_{ct}")
                    nc.scalar.dma_start(out=t[:], in_=wdram[ct])
                    w_sb[ci, ct] = t

            def load_x(img, eng=None):
                t = const.tile([128, 2, 1024], bf16, tag=f"xid_{img}",
                               name=f"xid_{img}")
                (eng or nc.sync).dma_start(out=t[:], in_=x_in[img])
                x_id[img] = t

            def zero_borders(t):
                # rows 0 and 33, plus cols 0 and 33 of rows 1..32 (one 2D AP)
                nc.vector.memset(t[:, 0:_PW], 0.0)
                nc.vector.memset(t[:, _PAD - _PW : _PAD], 0.0)
                base = t[:]
                colpair = bass.AP(
                    tensor=base.tensor,
                    offset=base.offset + _PW,
                    ap=[list(base.ap[0]), [_PW, 32], [33, 2]],
                )
                nc.vector.memset(colpair, 0.0)

            def prep(img):
                # build conv1 input (pad + cast to bf16); pre-zero conv2 input pads
                for ct in range(2):
                    t = const.tile([128, _PAD], bf16, tag=f"xp1_{img}_{ct}",
                                   name=f"xp1_{img}_{ct}")
                    zero_borders(t)
                    v = t[:].rearrange("p (y x) -> p y x", x=_PW)
                    xv = x_id[img][:, ct, :].rearrange("p (y x) -> p y x", x=32)
                    nc.vector.tensor_copy(v[:, 1:33, 1:33], xv)
                    xp[0, img, ct] = t
                    t2 = const.tile([128, _PAD], bf16, tag=f"xp2_{img}_{ct}",
                                    name=f"xp2_{img}_{ct}")
                    zero_borders(t2)
                    xp[1, img, ct] = t2

            def conv(img, ci, interleave=False, chunks=_CHUNKS, split_last_store=False):
                if ci == 1:
                    for ot in range(2):
                        outs[img, ot] = const.tile(
                            [128, 1024], f32, tag=f"out_{img}_{ot}",
                            name=f"out_{img}_{ot}",
                        )
                src_views = [
                    xp[ci, img, ct][:].rearrange("p (y x) -> p y x", x=_PW)
                    for ct in range(2)
                ]
                groups = []
                for ot in range(2):
                    for r0, nr in chunks:
                        ps = psum.tile([128, 512], f32, tag="ps", name="ps")
                        groups.append((ot, r0, nr, ps))

                def emit_mms(group_list):
                    for ct in range(2):
                        for ot, r0, nr, ps in group_list:
                            for tap in range(9):
                                dy, dx = tap // 3 - 1, tap % 3 - 1
                                rhs = src_views[ct][
                                    :, r0 + dy : r0 + dy + nr, 1 + dx : 33 + dx
                                ]
                                nc.tensor.matmul(
                                    ps[:, : nr * 32],
                                    w_sb[ci, ct][:, tap, ot, :],
                                    rhs,
                                    start=(ct == 0 and tap == 0),
                                    stop=(ct == 1 and tap == 8),
                                )

                if interleave:
                    # ct-outer across all 4 groups: gives the ct=1 data more
                    # time to arrive (used for the first conv only; it bunches
                    # the group completions, which is bad at the kernel tail).
                    emit_mms(groups)
                else:
                    for g in groups:
                        emit_mms([g])
                for ot, r0, nr, ps in groups:
                    n = nr * 32
                    pss = ps[:, :n]
                    if ci == 0:
                        dst = xp[1, img, ot][:].rearrange(
                            "p (y x) -> p y x", x=_PW
                        )
                        psv = pss.rearrange("p (y x) -> p y x", x=32)
                        nc.scalar.activation(
                            dst[:, r0 : r0 + nr, 1:33],
                            psv,
                            AF.Silu,
                            bias=sc_sb[:, 2 + ot : 3 + ot],
                            scale=sc_sb[:, ot : ot + 1],
                        )
                    else:
                        z = zpool.tile([128, 512], f32, tag="z", name="z")
                        zz = z[:, :n]
                        nc.vector.scalar_tensor_tensor(
                            out=zz,
                            in0=pss,
                            scalar=sc_sb[:, 4 + ot : 5 + ot],
                            in1=x_id[img][:, ot, (r0 - 1) * 32 : (r0 - 1 + nr) * 32],
                            op0=OP.mult,
                            op1=OP.add,
                        )
                        lo = (r0 - 1) * 32
                        nc.scalar.activation(
                            outs[img, ot][:, lo : lo + n],
                            zz,
                            AF.Silu,
                            bias=sc_sb[:, 6 + ot : 7 + ot],
                        )
                        is_last = (ot, r0, nr, ps) == groups[-1]
                        if split_last_store and is_last:
                            nc.sync.dma_start(
                                out=y_out[img, ot, :64, lo : lo + n],
                                in_=outs[img, ot][:64, lo : lo + n],
                            )
                            nc.scalar.dma_start(
                                out=y_out[img, ot, 64:, lo : lo + n],
                                in_=outs[img, ot][64:, lo : lo + n],
                            )
                        else:
                            eng = nc.sync if (r0 == 1) else nc.scalar
                            eng.dma_start(
                                out=y_out[img, ot, :, lo : lo + n],
                                in_=outs[img, ot][:, lo : lo + n],
                            )

            # critical-path-first emission: x(0) + conv1 weights first.
            # Queues: sync (HWDGE), scalar (HWDGE), gpsimd (SWDGE).
            # Critical loads: x0 (bf16, one full-packet DMA) on sync;
            # conv1 weights on scalar; scales (tiny packets) on gpsimd.
            load_x(0)
            load_w(0, w1p)
            # scales is 128 partitions x 32B = 128 tiny DMA packets (~4us of
            # queue time) -> keep it off the weight/activation critical queues
            nc.gpsimd.dma_start(out=sc_sb[:], in_=scl[:])

            # PE warm-up: trip the HAM clock-gate during the DMA wait so the
            # first real matmuls run at 2.4 GHz.
            warm = const.tile([128, 128], bf16, tag="warm", name="warm")
            nc.vector.memset(warm[:], 0.0)
            wps = psum.tile([128, 512], f32, tag="ps", name="wps")
            # ~5.4us of continuous cold PE activity: trips the free-running
            # HAM window so the real matmuls start at 2.4 GHz.
            for k in range(50):
                nc.tensor.matmul(wps[:, :128], warm[:], warm[:],
                                 start=(k == 0), stop=(k == 49))

            prep(0)
            load_x(1)
            load_w(1, w2p)
            load_x(2)
            load_x(3)
            prep(1)
            conv(0, 0, interleave=True)
            prep(2)
            conv(1, 0)
            conv(0, 1)
            prep(3)
            conv(2, 0)
            conv(1, 1)
            conv(3, 0)
            conv(2, 1)
            conv(3, 1, split_last_store=True)

    nc.finalize()
    return nc


_PROGRAM = None


def kernel(x, w1, b1, g1, be1, m1, v1, w2, b2, g2, be2, m2, v2):
    global LAST_RESULTS, _PROGRAM
    from concourse.bass_utils import run_bass_kernel_spmd

    import ml_dtypes

    x = np.asarray(x, np.float32)
    t1, s1 = _quantize(np.asarray(w1, np.float32))
    t2, s2 = _quantize(np.asarray(w2, np.float32))
    sc1, bb1 = _fold_bn(s1, np.asarray(b1), np.asarray(g1), np.asarray(be1),
                        np.asarray(m1), np.asarray(v1))
    sc2, bb2 = _fold_bn(s2, np.asarray(b2), np.asarray(g2), np.asarray(be2),
                        np.asarray(m2), np.asarray(v2))
    w1d = _prep_weights(t1)
    w2d = _prep_weights(t2)
    scales = np.ascontiguousarray(
        np.stack(
            [sc1[:128], sc1[128:], bb1[:128], bb1[128:],
             sc2[:128], sc2[128:], bb2[:128], bb2[128:]],
            axis=1,
        ).astype(np.float32)
    )

    if _PROGRAM is None:
        _PROGRAM = _build_program()
    nc = _PROGRAM

    n_cores = 8
    in_maps = []
    for i in range(n_cores):
        shard = np.ascontiguousarray(
            x[i * 4 : (i + 1) * 4]
            .reshape(4, 2, 128, 1024)
            .transpose(0, 2, 1, 3)
            .astype(ml_dtypes.bfloat16)
        )
        in_maps.append(
            {"x_in": shard, "w1p": w1d, "w2p": w2d, "scales": scales}
        )

    res = None
    for attempt in range(3):
        try:
            res = run_bass_kernel_spmd(nc, in_maps, list(range(n_cores)))
            break
        except Exception:
            if attempt == 2:
                raise
            import time

            time.sleep(2.0)
    LAST_RESULTS = res
    out = np.concatenate(
        [res.results[i]["y_out"].reshape(4, 256, 32, 32) for i in range(n_cores)],
        axis=0,
    )
    return out.astype(np.float32)


# revision 23
# speedup vs baseline: 1.0049x; 1.0035x over previous
"""Trainium2 Bass kernel: BitNet-style BasicBlock.

  out = silu(bn2(conv2(silu(bn1(conv1(x) )))) + x)

with ternary-quantized 3x3 conv weights (s = median|W|, Wq = clamp(round(W/s),-1,1)*s),
inference-mode BN folded into per-channel scale/bias.

Strategy:
  - Data-parallel over batch: 32 images -> 4 per NeuronCore (8 cores), weights replicated.
  - Conv3x3 via 9 shifted matmuls on a zero-padded 34x34 image layout. Weights enter the
    PE array as exact ternary {-1,0,+1} bf16; the quantization scale and BN affine are
    applied in f32 epilogues, so the only precision loss is bf16 rounding of activations.
  - Per output-channel-tile (128) and 16-row chunk, the moving operand is a strided
    [c, 16 rows, 32 cols] view (N=512, one PSUM bank); 18 matmuls (9 taps x 2 input
    channel tiles, K=128 each) accumulate into the bank at the PE stream roofline.
  - Epilogue conv1: ScalarE Silu(psum*sc1 + bb1) -> bf16 into the padded conv2 input.
    Epilogue conv2: VectorE (psum*sc2 + identity), ScalarE Silu(.+bb2) -> f32 out.
  - DMA queues: activations on qSync, weights on qScalar, stores on qGpSimd.
"""

import numpy as np

_BN_EPS = 1e-5

_PW = 34              # padded width/height
_PAD = _PW * _PW      # 1156
# output row chunks in padded coords: (first padded row, n rows)
_CHUNKS = [(1, 16), (17, 16)]

LAST_RESULTS = None   # BassKernelResults of the most recent run (for test harnesses)


def _quantize(w):
    """Ternary matrix (exact in bf16) + scale, matching the jax reference."""
    w = np.asarray(w, np.float32)
    s = np.median(np.abs(w))
    t = np.clip(np.round(w / s), -1.0, 1.0).astype(np.float32)
    return t, np.float32(s)


def _fold_bn(s, b, g, be, m, v):
    """BN(conv*s + b) = psum * sc + bb  (psum = ternary conv result)."""
    inv = g.astype(np.float32) / np.sqrt(v.astype(np.float32) + _BN_EPS)
    sc = (s * inv).astype(np.float32)
    bb = ((b - m) * inv + be).astype(np.float32)
    return sc, bb


def _prep_weights(t):
    """[O,C,3,3] ternary -> [ct, c128, tap, ot, o128] bf16 (lhsT layout)."""
    import ml_dtypes

    arr = t.transpose(1, 2, 3, 0).reshape(2, 128, 9, 2, 128)
    return np.ascontiguousarray(arr.astype(ml_dtypes.bfloat16))


def _build_program():
    import concourse.bass as bass
    import concourse.tile as tile
    from concourse import bacc, mybir

    f32 = mybir.dt.float32
    bf16 = mybir.dt.bfloat16
    AF = mybir.ActivationFunctionType
    OP = mybir.AluOpType

    nc = bacc.Bacc("TRN2", target_bir_lowering=False, debug=False,
                   enable_asserts=False, num_devices=8)
    x_in = nc.declare_dram_parameter("x_in", [4, 128, 2, 1024], bf16, isOutput=False)
    w1p = nc.declare_dram_parameter("w1p", [2, 128, 9, 2, 128], bf16, isOutput=False)
    w2p = nc.declare_dram_parameter("w2p", [2, 128, 9, 2, 128], bf16, isOutput=False)
    scl = nc.declare_dram_parameter("scales", [128, 8], f32, isOutput=False)
    y_out = nc.declare_dram_parameter("y_out", [4, 2, 128, 1024], f32, isOutput=True)

    with tile.TileContext(nc) as tc:
        from contextlib import ExitStack

        with ExitStack() as ctx:
            psum = ctx.enter_context(tc.tile_pool(name="psum", bufs=8, space="PSUM"))
            zpool = ctx.enter_context(tc.tile_pool(name="zp", bufs=4))
            const = ctx.enter_context(tc.tile_pool(name="const", bufs=1))

            w_sb = {}
            x_id = {}
            xp = {}      # (conv_input_index 0/1, img, ct) -> padded bf16 tile
            outs = {}

            sc_sb = const.tile([128, 8], f32, tag="scales", name="scales_sb")

            def load_w(ci, wdram):
                for ct in range(2):
                    t = const.tile([128, 9, 2, 128], bf16, tag=f"w{ci}_{ct}",
                                   name=f"w{ci}_{ct}")
                    nc.scalar.dma_start(out=t[:], in_=wdram[ct])
                    w_sb[ci, ct] = t

            def load_x(img, eng=None):
                t = const.tile([128, 2, 1024], bf16, tag=f"xid_{img}",
                               name=f"xid_{img}")
                (eng or nc.sync).dma_start(out=t[:], in_=x_in[img])
                x_id[img] = t

            def zero_borders(t):
                # rows 0 and 33, plus cols 0 and 33 of rows 1..32 (one 2D AP)
                nc.vector.memset(t[:, 0:_PW], 0.0)
                nc.vector.memset(t[:, _PAD - _PW : _PAD], 0.0)
                base = t[:]
                colpair = bass.AP(
                    tensor=base.tensor,
                    offset=base.offset + _PW,
                    ap=[list(base.ap[0]), [_PW, 32], [33, 2]],
                )
                nc.vector.memset(colpair, 0.0)

            def prep(img):
                # build conv1 input (pad + cast to bf16); pre-zero conv2 input pads
                for ct in range(2):
                    t = const.tile([128, _PAD], bf16, tag=f"xp1_{img}_{ct}",
                                   name=f"xp1_{img}_{ct}")
                    zero_borders(t)
                    v = t[:].rearrange("p (y x) -> p y x", x=_PW)
                    xv = x_id[img][:, ct, :].rearrange("p (y x) -> p y x", x=32)
                    nc.vector.tensor_copy(v[:, 1:33, 1:33], xv)
                    xp[0, img, ct] = t
                    t2 = const.tile([128, _PAD], bf16, tag=f"xp2_{img}_{ct}",
                                    name=f"xp2_{img}_{ct}")
                    zero_borders(t2)
                    xp[1, img, ct] = t2

            def conv(img, ci, interleave=False, chunks=_CHUNKS, split_last_store=False):
                if ci == 1:
                    for ot in range(2):
                        outs[img, ot] = const.tile(
                            [128, 1024], f32, tag=f"out_{img}_{ot}",
                            name=f"out_{img}_{ot}",
                        )
                src_views = [
                    xp[ci, img, ct][:].rearrange("p (y x) -> p y x", x=_PW)
                    for ct in range(2)
                ]
                groups = []
                for ot in range(2):
                    for r0, nr in chunks:
                        ps = psum.tile([128, 512], f32, tag="ps", name="ps")
                        groups.append((ot, r0, nr, ps))

                def emit_mms(group_list):
                    for ct in range(2):
                        for ot, r0, nr, ps in group_list:
                            for tap in range(9):
                                dy, dx = tap // 3 - 1, tap % 3 - 1
                                rhs = src_views[ct][
                                    :, r0 + dy : r0 + dy + nr, 1 + dx : 33 + dx
                                ]
                                nc.tensor.matmul(
                                    ps[:, : nr * 32],
                                    w_sb[ci, ct][:, tap, ot, :],
                                    rhs,
                                    start=(ct == 0 and tap == 0),
                                    stop=(ct == 1 and tap == 8),
                                )

                if interleave:
                    # ct-outer across all 4 groups: gives the ct=1 data more
                    # time to arrive (used for the first conv only; it bunches
                    # the group completions, which is bad at the kernel tail).
                    emit_mms(groups)
                else:
                    for g in groups:
                        emit_mms([g])
                for ot, r0, nr, ps in groups:
                    n = nr * 32
                    pss = ps[:, :n]
                    if ci == 0:
                        dst = xp[1, img, ot][:].rearrange(
                            "p (y x) -> p y x", x=_PW
                        )
                        psv = pss.rearrange("p (y x) -> p y x", x=32)
                        nc.scalar.activation(
                            dst[:, r0 : r0 + nr, 1:33],
                            psv,
                            AF.Silu,
                            bias=sc_sb[:, 2 + ot : 3 + ot],
                            scale=sc_sb[:, ot : ot + 1],
                        )
                    else:
                        z = zpool.tile([128, 512], f32, tag="z", name="z")
                        zz = z[:, :n]
                        nc.vector.scalar_tensor_tensor(
                            out=zz,
                            in0=pss,
                            scalar=sc_sb[:, 4 + ot : 5 + ot],
                            in1=x_id[img][:, ot, (r0 - 1) * 32 : (r0 - 1 + nr) * 32],
                            op0=OP.mult,
                            op1=OP.add,
                        )
                        lo = (r0 - 1) * 32
                        nc.scalar.activation(
                            outs[img, ot][:, lo : lo + n],
                            zz,
                            AF.Silu,
                            bias=sc_sb[:, 6 + ot : 7 + ot],
                        )
                        is_last = (ot, r0, nr, ps) == groups[-1]
                        if split_last_store and is_last:
                            nc.sync.dma_start(
                                out=y_out[img, ot, :64, lo : lo + n],
                                in_=outs[img, ot][:64, lo : lo + n],
                            )
                            nc.scalar.dma_start(
                                out=y_out[img, ot, 64:, lo : lo + n],
                                in_=outs[img, ot][64:, lo : lo + n],
                            )
                        else:
                            eng = nc.sync if (r0 == 1) else nc.scalar
                            eng.dma_start(
                                out=y_out[img, ot, :, lo : lo + n],
                                in_=outs[img, ot][:, lo : lo + n],
                            )

            # critical-path-first emission: x(0) + conv1 weights first.
            # Queues: sync (HWDGE), scalar (HWDGE), gpsimd (SWDGE).
            # Critical loads: x0 (bf16, one full-packet DMA) on sync;
            # conv1 weights on scalar; scales (tiny packets) on gpsimd.
            load_x(0)
            load_w(0, w1p)
            # scales is 128 partitions x 32B = 128 tiny DMA packets (~4us of
            # queue time) -> keep it off the weight/activation critical queues
            nc.gpsimd.dma_start(out=sc_sb[:], in_=scl[:])

            # PE warm-up: trip the HAM clock-gate during the DMA wait so the
            # first real matmuls run at 2.4 GHz.
            warm = const.tile([128, 128], bf16, tag="warm", name="warm")
            nc.vector.memset(warm[:], 0.0)
            wps = psum.tile([128, 512], f32, tag="ps", name="wps")
            # ~5.4us of continuous cold PE activity: trips the free-running
            # HAM window so the real matmuls start at 2.4 GHz.
            for k in range(50):
                nc.tensor.matmul(wps[:, :128], warm[:], warm[:],
                                 start=(k == 0), stop=(k == 49))

            prep(0)
            load_x(1)
            load_w(1, w2p)
            load_x(2)
            load_x(3)
            prep(1)
            conv(0, 0, interleave=True)
            prep(2)
            conv(1, 0)
            conv(0, 1)
            prep(3)
            conv(2, 0)
            conv(1, 1)
            conv(3, 0)
            conv(2, 1)
            conv(3, 1)

    nc.finalize()
    return nc


_PROGRAM = None


def kernel(x, w1, b1, g1, be1, m1, v1, w2, b2, g2, be2, m2, v2):
    global LAST_RESULTS, _PROGRAM
    from concourse.bass_utils import run_bass_kernel_spmd

    import ml_dtypes

    x = np.asarray(x, np.float32)
    t1, s1 = _quantize(np.asarray(w1, np.float32))
    t2, s2 = _quantize(np.asarray(w2, np.float32))
    sc1, bb1 = _fold_bn(s1, np.asarray(b1), np.asarray(g1), np.asarray(be1),
                        np.asarray(m1), np.asarray(v1))
    sc2, bb2 = _fold_bn(s2, np.asarray(b2), np.asarray(g2), np.asarray(be2),
                        np.asarray(m2), np.asarray(v2))
    w1d = _prep_weights(t1)
    w2d = _prep_weights(t2)
    scales = np.ascontiguousarray(
        np.stack(
            [sc1[:128], sc1[128:], bb1[:128], bb1[128:],
             sc2[:128], sc2[128:], bb2[:128], bb2[128:]],
            axis=1,
        ).astype(np.float32)
    )

    if _PROGRAM is None:
        _PROGRAM = _build_program()
    nc = _PROGRAM

    n_cores = 8
    in_maps = []
    for i in range(n_cores):
        shard = np.ascontiguousarray(
            x[i * 4 : (i + 1) * 4]
            .reshape(4, 2, 128, 1024)
            .transpose(0, 2, 1, 3)
            .astype(ml_dtypes.bfloat16)
        )
        in_maps.append(
            {"x_in": shard, "w1p": w1d, "w2p": w2d, "scales": scales}
        )

    def _poke_devices():
        # A small op per device clears transient wedged NRT state (observed:
        # a failed prior process can leave the next execute unrecoverable
        # until something simple runs on the core).
        try:
            import jax
            import jax.numpy as jnp

            for dev in jax.devices()[:n_cores]:
                jnp.add(jax.device_put(jnp.ones((8, 8)), dev), 1.0).block_until_ready()
        except Exception:
            pass

    res = None
    for attempt in range(3):
        try:
            res = run_bass_kernel_spmd(nc, in_maps, list(range(n_cores)))
            break
        except Exception:
            if attempt == 2:
                raise
            import time

            time.sleep(2.0)
            _poke_devices()
    LAST_RESULTS = res
    out = np.concatenate(
        [res.results[i]["y_out"].reshape(4, 256, 32, 32) for i in range(n_cores)],
        axis=0,
    )
    return out.astype(np.float32)


# revision 24
# speedup vs baseline: 1.0101x; 1.0052x over previous
"""Trainium2 Bass kernel: BitNet-style BasicBlock.

  out = silu(bn2(conv2(silu(bn1(conv1(x) )))) + x)

with ternary-quantized 3x3 conv weights (s = median|W|, Wq = clamp(round(W/s),-1,1)*s),
inference-mode BN folded into per-channel scale/bias.

Strategy:
  - Data-parallel over batch: 32 images -> 4 per NeuronCore (8 cores), weights replicated.
  - Conv3x3 via 9 shifted matmuls on a zero-padded 34x34 image layout. Weights enter the
    PE array as exact ternary {-1,0,+1} bf16; the quantization scale and BN affine are
    applied in f32 epilogues, so the only precision loss is bf16 rounding of activations.
  - Per output-channel-tile (128) and 16-row chunk, the moving operand is a strided
    [c, 16 rows, 32 cols] view (N=512, one PSUM bank); 18 matmuls (9 taps x 2 input
    channel tiles, K=128 each) accumulate into the bank at the PE stream roofline.
  - Epilogue conv1: ScalarE Silu(psum*sc1 + bb1) -> bf16 into the padded conv2 input.
    Epilogue conv2: VectorE (psum*sc2 + identity), ScalarE Silu(.+bb2) -> f32 out.
  - DMA queues: activations on qSync, weights on qScalar, stores on qGpSimd.
"""

import numpy as np

_BN_EPS = 1e-5

_PW = 34              # padded width/height
_PAD = _PW * _PW      # 1156
# output row chunks in padded coords: (first padded row, n rows)
_CHUNKS = [(1, 16), (17, 16)]

LAST_RESULTS = None   # BassKernelResults of the most recent run (for test harnesses)


def _quantize(w):
    """Ternary matrix (exact in bf16) + scale, matching the jax reference."""
    w = np.asarray(w, np.float32)
    s = np.median(np.abs(w))
    t = np.clip(np.round(w / s), -1.0, 1.0).astype(np.float32)
    return t, np.float32(s)


def _fold_bn(s, b, g, be, m, v):
    """BN(conv*s + b) = psum * sc + bb  (psum = ternary conv result)."""
    inv = g.astype(np.float32) / np.sqrt(v.astype(np.float32) + _BN_EPS)
    sc = (s * inv).astype(np.float32)
    bb = ((b - m) * inv + be).astype(np.float32)
    return sc, bb


def _prep_weights(t):
    """[O,C,3,3] ternary -> [ct, c128, tap, ot, o128] bf16 (lhsT layout)."""
    import ml_dtypes

    arr = t.transpose(1, 2, 3, 0).reshape(2, 128, 9, 2, 128)
    return np.ascontiguousarray(arr.astype(ml_dtypes.bfloat16))


def _build_program():
    import concourse.bass as bass
    import concourse.tile as tile
    from concourse import bacc, mybir

    f32 = mybir.dt.float32
    bf16 = mybir.dt.bfloat16
    AF = mybir.ActivationFunctionType
    OP = mybir.AluOpType

    nc = bacc.Bacc("TRN2", target_bir_lowering=False, debug=False,
                   enable_asserts=False, num_devices=8)
    x_in = nc.declare_dram_parameter("x_in", [4, 128, 2, 1024], bf16, isOutput=False)
    w1p = nc.declare_dram_parameter("w1p", [2, 128, 9, 2, 128], bf16, isOutput=False)
    w2p = nc.declare_dram_parameter("w2p", [2, 128, 9, 2, 128], bf16, isOutput=False)
    scl = nc.declare_dram_parameter("scales", [128, 8], f32, isOutput=False)
    y_out = nc.declare_dram_parameter("y_out", [4, 2, 128, 1024], f32, isOutput=True)

    import types

    from concourse.vector_clock import ScopedClock

    def _lean_drain_and_barrier(self, tick_clock, wait_clock):
        # Same as TileContext._drain_and_barrier minus the final all-engine
        # barrier: it only holds the other engines until gpsimd's sem-range
        # clear finishes, but NEFF completion already waits for every engine,
        # and the Bass postamble touches a disjoint sem range.
        drain_inst = self.nc.sync.drain()
        wait_clock.add_sem_waits(
            drain_inst.ins, ScopedClock({None: tick_clock.global_clock})
        )
        self.nc.all_engine_barrier()
        assert self.sems is not None
        popped = self.nc._tile_sem_poison_stack.pop()
        assert popped is self._sem_poison
        self.nc.clear_and_free_semaphores(list(self.sems.allocated().values()))

    with tile.TileContext(nc) as tc:
        tc._drain_and_barrier = types.MethodType(_lean_drain_and_barrier, tc)
        from contextlib import ExitStack

        with ExitStack() as ctx:
            psum = ctx.enter_context(tc.tile_pool(name="psum", bufs=8, space="PSUM"))
            zpool = ctx.enter_context(tc.tile_pool(name="zp", bufs=4))
            const = ctx.enter_context(tc.tile_pool(name="const", bufs=1))

            w_sb = {}
            x_id = {}
            xp = {}      # (conv_input_index 0/1, img, ct) -> padded bf16 tile
            outs = {}

            sc_sb = const.tile([128, 8], f32, tag="scales", name="scales_sb")

            def load_w(ci, wdram):
                for ct in range(2):
                    t = const.tile([128, 9, 2, 128], bf16, tag=f"w{ci}_{ct}",
                                   name=f"w{ci}_{ct}")
                    nc.scalar.dma_start(out=t[:], in_=wdram[ct])
                    w_sb[ci, ct] = t

            def load_x(img, eng=None):
                t = const.tile([128, 2, 1024], bf16, tag=f"xid_{img}",
                               name=f"xid_{img}")
                (eng or nc.sync).dma_start(out=t[:], in_=x_in[img])
                x_id[img] = t

            def zero_borders(t):
                # rows 0 and 33, plus cols 0 and 33 of rows 1..32 (one 2D AP)
                nc.vector.memset(t[:, 0:_PW], 0.0)
                nc.vector.memset(t[:, _PAD - _PW : _PAD], 0.0)
                base = t[:]
                colpair = bass.AP(
                    tensor=base.tensor,
                    offset=base.offset + _PW,
                    ap=[list(base.ap[0]), [_PW, 32], [33, 2]],
                )
                nc.vector.memset(colpair, 0.0)

            def prep(img):
                # build conv1 input (pad + cast to bf16); pre-zero conv2 input pads
                for ct in range(2):
                    t = const.tile([128, _PAD], bf16, tag=f"xp1_{img}_{ct}",
                                   name=f"xp1_{img}_{ct}")
                    zero_borders(t)
                    v = t[:].rearrange("p (y x) -> p y x", x=_PW)
                    xv = x_id[img][:, ct, :].rearrange("p (y x) -> p y x", x=32)
                    nc.vector.tensor_copy(v[:, 1:33, 1:33], xv)
                    xp[0, img, ct] = t
                    t2 = const.tile([128, _PAD], bf16, tag=f"xp2_{img}_{ct}",
                                    name=f"xp2_{img}_{ct}")
                    zero_borders(t2)
                    xp[1, img, ct] = t2

            def conv(img, ci, interleave=False, chunks=_CHUNKS, split_last_store=False):
                if ci == 1:
                    for ot in range(2):
                        outs[img, ot] = const.tile(
                            [128, 1024], f32, tag=f"out_{img}_{ot}",
                            name=f"out_{img}_{ot}",
                        )
                src_views = [
                    xp[ci, img, ct][:].rearrange("p (y x) -> p y x", x=_PW)
                    for ct in range(2)
                ]
                groups = []
                for ot in range(2):
                    for r0, nr in chunks:
                        ps = psum.tile([128, 512], f32, tag="ps", name="ps")
                        groups.append((ot, r0, nr, ps))

                def emit_mms(group_list):
                    for ct in range(2):
                        for ot, r0, nr, ps in group_list:
                            for tap in range(9):
                                dy, dx = tap // 3 - 1, tap % 3 - 1
                                rhs = src_views[ct][
                                    :, r0 + dy : r0 + dy + nr, 1 + dx : 33 + dx
                                ]
                                nc.tensor.matmul(
                                    ps[:, : nr * 32],
                                    w_sb[ci, ct][:, tap, ot, :],
                                    rhs,
                                    start=(ct == 0 and tap == 0),
                                    stop=(ct == 1 and tap == 8),
                                )

                if interleave:
                    # ct-outer across all 4 groups: gives the ct=1 data more
                    # time to arrive (used for the first conv only; it bunches
                    # the group completions, which is bad at the kernel tail).
                    emit_mms(groups)
                else:
                    for g in groups:
                        emit_mms([g])
                for ot, r0, nr, ps in groups:
                    n = nr * 32
                    pss = ps[:, :n]
                    if ci == 0:
                        dst = xp[1, img, ot][:].rearrange(
                            "p (y x) -> p y x", x=_PW
                        )
                        psv = pss.rearrange("p (y x) -> p y x", x=32)
                        nc.scalar.activation(
                            dst[:, r0 : r0 + nr, 1:33],
                            psv,
                            AF.Silu,
                            bias=sc_sb[:, 2 + ot : 3 + ot],
                            scale=sc_sb[:, ot : ot + 1],
                        )
                    else:
                        z = zpool.tile([128, 512], f32, tag="z", name="z")
                        zz = z[:, :n]
                        nc.vector.scalar_tensor_tensor(
                            out=zz,
                            in0=pss,
                            scalar=sc_sb[:, 4 + ot : 5 + ot],
                            in1=x_id[img][:, ot, (r0 - 1) * 32 : (r0 - 1 + nr) * 32],
                            op0=OP.mult,
                            op1=OP.add,
                        )
                        lo = (r0 - 1) * 32
                        nc.scalar.activation(
                            outs[img, ot][:, lo : lo + n],
                            zz,
                            AF.Silu,
                            bias=sc_sb[:, 6 + ot : 7 + ot],
                        )
                        is_last = (ot, r0, nr, ps) == groups[-1]
                        if split_last_store and is_last:
                            nc.sync.dma_start(
                                out=y_out[img, ot, :64, lo : lo + n],
                                in_=outs[img, ot][:64, lo : lo + n],
                            )
                            nc.scalar.dma_start(
                                out=y_out[img, ot, 64:, lo : lo + n],
                                in_=outs[img, ot][64:, lo : lo + n],
                            )
                        else:
                            eng = nc.sync if (r0 == 1) else nc.scalar
                            eng.dma_start(
                                out=y_out[img, ot, :, lo : lo + n],
                                in_=outs[img, ot][:, lo : lo + n],
                            )

            # critical-path-first emission: x(0) + conv1 weights first.
            # Queues: sync (HWDGE), scalar (HWDGE), gpsimd (SWDGE).
            # Critical loads: x0 (bf16, one full-packet DMA) on sync;
            # conv1 weights on scalar; scales (tiny packets) on gpsimd.
            load_x(0)
            load_w(0, w1p)
            # scales is 128 partitions x 32B = 128 tiny DMA packets (~4us of
            # queue time) -> keep it off the weight/activation critical queues
            nc.gpsimd.dma_start(out=sc_sb[:], in_=scl[:])

            # PE warm-up: trip the HAM clock-gate during the DMA wait so the
            # first real matmuls run at 2.4 GHz.
            warm = const.tile([128, 128], bf16, tag="warm", name="warm")
            nc.vector.memset(warm[:], 0.0)
            wps = psum.tile([128, 512], f32, tag="ps", name="wps")
            # ~5.4us of continuous cold PE activity: trips the free-running
            # HAM window so the real matmuls start at 2.4 GHz.
            for k in range(50):
                nc.tensor.matmul(wps[:, :128], warm[:], warm[:],
                                 start=(k == 0), stop=(k == 49))

            prep(0)
            load_x(1)
            load_w(1, w2p)
            load_x(2)
            load_x(3)
            prep(1)
            conv(0, 0, interleave=True)
            prep(2)
            conv(1, 0)
            conv(0, 1)
            prep(3)
            conv(2, 0)
            conv(1, 1)
            conv(3, 0)
            conv(2, 1)
            conv(3, 1)

    nc.finalize()
    return nc


_PROGRAM = None


def kernel(x, w1, b1, g1, be1, m1, v1, w2, b2, g2, be2, m2, v2):
    global LAST_RESULTS, _PROGRAM
    from concourse.bass_utils import run_bass_kernel_spmd

    import ml_dtypes

    x = np.asarray(x, np.float32)
    t1, s1 = _quantize(np.asarray(w1, np.float32))
    t2, s2 = _quantize(np.asarray(w2, np.float32))
    sc1, bb1 = _fold_bn(s1, np.asarray(b1), np.asarray(g1), np.asarray(be1),
                        np.asarray(m1), np.asarray(v1))
    sc2, bb2 = _fold_bn(s2, np.asarray(b2), np.asarray(g2), np.asarray(be2),
                        np.asarray(m2), np.asarray(v2))
    w1d = _prep_weights(t1)
    w2d = _prep_weights(t2)
    scales = np.ascontiguousarray(
        np.stack(
            [sc1[:128], sc1[128:], bb1[:128], bb1[128:],
             sc2[:128], sc2[128:], bb2[:128], bb2[128:]],
            axis=1,
        ).astype(np.float32)
    )

    if _PROGRAM is None:
        _PROGRAM = _build_program()
    nc = _PROGRAM

    n_cores = 8
    in_maps = []
    for i in range(n_cores):
        shard = np.ascontiguousarray(
            x[i * 4 : (i + 1) * 4]
            .reshape(4, 2, 128, 1024)
            .transpose(0, 2, 1, 3)
            .astype(ml_dtypes.bfloat16)
        )
        in_maps.append(
            {"x_in": shard, "w1p": w1d, "w2p": w2d, "scales": scales}
        )

    def _poke_devices():
        # A small op per device clears transient wedged NRT state (observed:
        # a failed prior process can leave the next execute unrecoverable
        # until something simple runs on the core).
        try:
            import jax
            import jax.numpy as jnp

            for dev in jax.devices()[:n_cores]:
                jnp.add(jax.device_put(jnp.ones((8, 8)), dev), 1.0).block_until_ready()
        except Exception:
            pass

    res = None
    for attempt in range(3):
        try:
            res = run_bass_kernel_spmd(nc, in_maps, list(range(n_cores)))
            break
        except Exception:
            if attempt == 2:
                raise
            import time

            time.sleep(2.0)
            _poke_devices()
    LAST_RESULTS = res
    out = np.concatenate(
        [res.results[i]["y_out"].reshape(4, 256, 32, 32) for i in range(n_cores)],
        axis=0,
    )
    return out.astype(np.float32)
